# revision 81
# baseline (speedup 1.0000x reference)
"""Multi-head attention (D=2048, H=16, B=2, S=2048, causal, RoPE) on 8 TRN2 cores.

Sharding: tensor-parallel over heads -- 2 heads per core, both batches.
Each core computes q/k/v projections for its 2 heads, RoPE, causal flash-style
attention, and a partial output projection over its heads' columns of wo.
The host sums the 8 partial outputs (the out-projection contracts over heads,
which is the sharded axis).

Schedule (KILV=5, emission-interleaved): attention unit (b, qc) runs as soon
as projection chunk b*4+qc exists; the NEXT projection chunk's matmuls and
the previous unit's out-projection are woven into its attention steps as
~1us PE quanta (between scores and AV, hiding the ACT exp latency). This
keeps PE >99% dense mid-kernel -- critical because any PE gap >100ns drops
the tensor clock from 2.4GHz to 1.2GHz for the next ~3us of work.

Key engine placement:
- softmax denominator: bf16 running sum of exp tiles on DVE (KDEN=dve), one
  f32 ones-matmul per chunk for the 128-way partition reduction (which also
  gives the partition broadcast for free). Removes 160 PE matmuls (~37us).
- RoPE rotation: DVE stream_shuffle lane-swap + sign-baked sin table
  (KRSH=1) instead of a PE rotation matmul. Removes 32 PE matmuls.
- chunk finishes (den-reduce/recip/normalize) are deferred into the next
  chunk (KFIN=1) so PE never waits on the DVE den drain.
- PSUM rings: pp(1) + ps(2: outproj/rot/warm) + out(2: att out, so deferred
  finishes never pin the shared ring) + pss(3: scores/den) = 8 banks.
- startup: per-d-tile xt/wq DMA pieces in consumption order, cos/sin sliced,
  wv split across both ~100GB/s engine rings; dummy warmup matmuls (+pads,
  KSPAD) hold the p-state while supply streams. Output DMAs alternate
  sync/scalar rings so the xt stream never queues behind output bursts.
- tail: the last finish normalizes per q-subtile and launches that subtile's
  outproj immediately; tail evictions/DMAs alternate ACT/DVE and both rings.

Output partials are written bf16 (host sums 8 partials in f32): halves
output DMA bytes for ~3e-4 extra max-rel error.

Self-contained: hardcodes all shapes; only needs numpy/ml_dtypes/concourse.
"""
import os
import sys
import time

for _p in ("/opt/trn_rl_repo",):
    if os.path.isdir(_p) and _p not in sys.path:
        sys.path.append(_p)

import numpy as np
import ml_dtypes
from contextlib import ExitStack

import concourse.bass as bass
import concourse.tile as tile
from concourse import bacc, mybir

BF = mybir.dt.bfloat16
F32 = mybir.dt.float32
F32R = mybir.dt.float32r
BF_NP = ml_dtypes.bfloat16

B = 2
S = 2048
D = 2048
H = 16
HD = 128  # head dim
N_CORES = 8
H_CORE = H // N_CORES          # heads per core = 2
E = H_CORE * HD                # per-core q/k/v width = 256
BS = B * S                     # 4096 flattened tokens
P = 128
SC = 512                       # s-chunk (free dim of projection matmuls)
N_SC = BS // SC                # 8 s-chunks
N_DT = D // P                  # 16 d-tiles (contraction)
QC = 512                       # q-chunk in attention
N_QC = S // QC                 # 4 q-chunks per (batch, head)
N_KT = S // P                  # 16 k-tiles per (batch, head)
SCALE = 1.0 / float(np.sqrt(HD))
ROPE_BASE = 10000.0
# stream_shuffle mask: swap lanes within each even/odd pair (32-lane groups)
SWAP_MASK = [i ^ 1 for i in range(32)]


def _build_program():
    """Build the per-core Bass program (identical on all cores; data differs)."""
    nc = bacc.Bacc("TRN2", target_bir_lowering=False, debug=False)

    # all big inputs are host-packed to the exact SBUF layout so every DMA is
    # one long contiguous run per partition (few descriptors, fast HWDGE)
    xt_d = nc.dram_tensor("xt", [N_SC, P, N_DT * SC], BF, kind="ExternalInput").ap()
    wqt_d = nc.dram_tensor("wqt", [P, N_DT * E], BF, kind="ExternalInput").ap()
    wkt_d = nc.dram_tensor("wkt", [P, N_DT * E], BF, kind="ExternalInput").ap()
    wvt_d = nc.dram_tensor("wvt", [P, N_DT * E], BF, kind="ExternalInput").ap()
    wot_d = nc.dram_tensor("wot", [P, H_CORE * D], BF, kind="ExternalInput").ap()
    cos_d = nc.dram_tensor("cos", [P, S], BF, kind="ExternalInput").ap()
    sin_d = nc.dram_tensor("sin", [P, S], BF, kind="ExternalInput").ap()
    sins_d = nc.dram_tensor("sins", [P, S], BF, kind="ExternalInput").ap()
    rmat_d = nc.dram_tensor("rmat", [P, P], BF, kind="ExternalInput").ap()
    tri_d = nc.dram_tensor("tri", [P, P], BF, kind="ExternalInput").ap()
    obf = os.environ.get("KOBF", "1") == "1"
    out_d = nc.dram_tensor("out", [BS, D], BF if obf else F32,
                           kind="ExternalOutput").ap()

    with tile.TileContext(nc) as tc:
        with ExitStack() as ctx:
            _emit(ctx, tc, nc, xt_d, wqt_d, wkt_d, wvt_d, wot_d,
                  cos_d, sin_d, sins_d, rmat_d, tri_d, out_d)
    nc.compile()
    return nc


def _emit(ctx, tc, nc, xt_d, wqt_d, wkt_d, wvt_d, wot_d,
          cos_d, sin_d, sins_d, rmat_d, tri_d, out_d):
    Exp = mybir.ActivationFunctionType.Exp
    # NOTE: Pool (gpsimd) measures ~1155ns per [128,512] tensor op on hw --
    # ~3x the DVE-class model -- so accumulating softmax denominators there
    # stalls PE ~3.8us per chunk. DVE measures ~134-250ns for a bf16 add, so
    # den lives there (KDEN=dve): acc += at per k-tile, one ones-matmul per
    # chunk for the partition reduction. Each acc element sums <=16 bf16
    # values so the rounding is ~0.5% on acc, ~0.04% after the 128-way f32
    # partition sum -- negligible vs the 2e-2 gate.
    den_mode = os.environ.get("KDEN", "dve")
    use_pool_den = den_mode == "pool"
    use_dve_den = den_mode == "dve"
    # KRSH=1: RoPE rotation via a DVE stream_shuffle (swap lanes within each
    # even/odd pair) + sign-baked sin table, instead of a PE matmul with the
    # rotation matrix: saves 32 PE matmuls (~7.7us) and all rot PSUM traffic
    krsh = os.environ.get("KRSH", "1") == "1"
    kfin = os.environ.get("KFIN", "1") == "1"
    kilv = os.environ.get("KILV", "1")
    kwarm = int(os.environ.get("KWARM", "4"))
    xtq_scalar = os.environ.get("KXTQ", "sync") == "scalar"

    const = ctx.enter_context(tc.tile_pool(name="const", bufs=1))
    xpool = ctx.enter_context(tc.tile_pool(name="xpool", bufs=int(os.environ.get("KXP","2"))))
    qkv = ctx.enter_context(tc.tile_pool(name="qkv", bufs=1))
    rope = ctx.enter_context(tc.tile_pool(name="rope", bufs=int(os.environ.get("KROPE","4"))))
    att = ctx.enter_context(tc.tile_pool(name="att", bufs=8))
    nrm = ctx.enter_context(tc.tile_pool(name="nrm", bufs=int(os.environ.get("KNRM","4"))))
    den = ctx.enter_context(tc.tile_pool(name="den", bufs=2))
    outp = ctx.enter_context(tc.tile_pool(name="outp", bufs=int(os.environ.get("KOUTP","16"))))
    psum = ctx.enter_context(tc.tile_pool(name="psum", bufs=int(os.environ.get("KACC","2")), space="PSUM"))
    psum_s = ctx.enter_context(tc.tile_pool(name="psum_s", bufs=int(os.environ.get("KSTR","3")), space="PSUM"))

    # ---- constants / weights in SBUF ----
    # input queue: xt rides one HWDGE queue, weights the other, outputs go to
    # the weights' queue later (sync) so the xt stream never queues behind
    # output bursts
    xt_eng = nc.scalar if xtq_scalar else nc.sync
    w_eng = nc.sync if xtq_scalar else nc.scalar

    xt_c0 = xpool.tile([P, N_DT * SC], BF, tag="xt")
    wq_sb = const.tile([P, N_DT * E], BF)
    wk_sb = const.tile([P, N_DT * E], BF)
    wv_sb = const.tile([P, N_DT * E], BF)
    rmat_sb = const.tile([P, P], BF)
    tri_sb = const.tile([P, P], BF)
    cos_sb = const.tile([P, S], BF)
    sin_sb = const.tile([P, S], BF)
    wo_sb = const.tile([P, H_CORE * D], BF)
    # Startup supply schedule. Each dma_start costs ~620ns of issue time on
    # its engine queue regardless of size, so: per-d-tile pieces up front
    # (PE's first pass over xt0 consumes a tile per ~430-550ns and follows
    # the supply), then coarse pieces that stream ahead of consumption.
    # cos/sin are sliced so only chunk 0's 512 columns sit on the critical
    # path; tri/wo/cos-rest land long before their first (late) use.
    xt_pieces = [(0, 1), (1, 2), (2, 3), (3, 4), (4, 6), (6, 9), (9, 16)]
    for t0_, t1_ in xt_pieces:
        xt_eng.dma_start(xt_c0[:, t0_ * SC:t1_ * SC],
                         xt_d[0][:, t0_ * SC:t1_ * SC])
    wq_pieces = [(0, 1), (1, 2), (2, 4), (4, 16)]
    for i, (t0_, t1_) in enumerate(wq_pieces):
        w_eng.dma_start(wq_sb[:, t0_ * E:t1_ * E],
                        wqt_d[:, t0_ * E:t1_ * E])
        if i == 0:
            w_eng.dma_start(rmat_sb[:], rmat_d[:])
    sin_src = sins_d if krsh else sin_d
    # K/V weights and late-need constants ride the otherwise-idle gpsimd
    # SWDGE ring (KGPD=1) so the scalar ring only carries wq + rope chunk-0
    # tables -- halves the startup supply serialization
    kv_eng = nc.gpsimd if os.environ.get("KGPD", "0") == "1" else w_eng
    kv_eng.dma_start(wk_sb[:, :4 * E], wkt_d[:, :4 * E])
    w_eng.dma_start(cos_sb[:, :SC], cos_d[:, :SC])
    w_eng.dma_start(sin_sb[:, :SC], sin_src[:, :SC])
    kv_eng.dma_start(wk_sb[:, 4 * E:8 * E], wkt_d[:, 4 * E:8 * E])
    kv_eng.dma_start(wk_sb[:, 8 * E:], wkt_d[:, 8 * E:])
    # each engine ring moves ~105GB/s: balance startup bytes by putting half
    # of wv on the xt (sync) ring, which finishes its 2MB before V-proj needs
    # weights; the scalar ring keeps ~2.25MB
    wv2_eng = xt_eng if os.environ.get("KWVS", "1") == "1" else kv_eng
    kv_eng.dma_start(wv_sb[:, :8 * E], wvt_d[:, :8 * E])
    wv2_eng.dma_start(wv_sb[:, 8 * E:], wvt_d[:, 8 * E:])
    w_eng.dma_start(tri_sb[:], tri_d[:])
    w_eng.dma_start(cos_sb[:, SC:], cos_d[:, SC:])
    w_eng.dma_start(sin_sb[:, SC:], sin_src[:, SC:])
    ones_sb = const.tile([P, P], BF)
    # memset on Pool: it is the earliest-active engine (the framework's own
    # const-AP memsets run there in the preamble), so the PE warmup below can
    # start ~2us sooner than if gated on DVE's slower preamble
    warm_eng = nc.gpsimd if os.environ.get("KWME", "vector") == "pool" else nc.vector
    warm_eng.memset(ones_sb[:], 1.0)
    # wot in [128, 2 * D] packed layout; needed only for out-projection
    w_eng.dma_start(wo_sb[:], wot_d[:])

    # PE warmup: dummy matmuls ramp the tensor-engine p-state clock while the
    # first xt/weight DMAs are still streaming in
    warm_sb = warm_ps = None
    warm_closed = [False]
    if kwarm:
        warm_sb = const.tile([P, SC], BF)
        warm_eng.memset(warm_sb[:], 0.0)
        warm_ps = psum.tile([P, SC], F32, tag="ps")
        for i in range(kwarm):
            nc.tensor.matmul(warm_ps[:], ones_sb[:], warm_sb[:],
                             start=True, stop=True)
        # reader IMMEDIATELY after the warmup writes: a warm_ps that stays
        # live while rot/po cycle the "ps" ring is a timing-dependent
        # corruption race (the pool does not protect slot reuse against a
        # late reader). No pads between phase1 quanta for the same reason.
        nc.vector.tensor_copy(warm_sb[:], warm_ps[:])
        warm_closed[0] = True

    def warm_pad(n):
        # dummy matmuls between startup quanta: the startup window is DMA-BW
        # bound, so these execute in time PE would idle anyway -- and they
        # hold the p-state at max so the real matmuls run 216ns, not 427ns
        for _ in range(n):
            nc.tensor.matmul(warm_ps[:], ones_sb[:], warm_sb[:],
                             start=True, stop=True)

    def warm_close():
        # give the warmup PSUM a reader (BIR verifier requires one; Pool
        # cannot read PSUM, so use DVE -- idle at startup)
        if warm_ps is not None and not warm_closed[0]:
            warm_closed[0] = True
            nc.vector.tensor_copy(warm_sb[:], warm_ps[:])

    # persistent activations
    qT = qkv.tile([P, H_CORE * BS], BF)   # [d, (head, b*s)] rope'd q
    kT = qkv.tile([P, H_CORE * BS], BF)   # [d, (head, b*s)] rope'd k
    v_sb = qkv.tile([P, (BS // P) * E], BF)  # [s within tile, (s-tile, e)]
    aoT = qkv.tile([P, H_CORE * BS], BF)  # [d, (b, head, q)] normalized attn out

    # ---- phase 1: projections + RoPE ----
    # Generator form: yields after ~1us PE quanta so the driver can weave
    # projection matmuls into attention steps (fills the PE slack created by
    # ACT's exp pacing and keeps the PE clock at max p-state).
    def phase1_gen(sc, xt_c):
        b = sc // (N_SC // B)
        s_lo = (sc % (N_SC // B)) * SC  # within-batch s offset

        # qT / kT (with RoPE) per head (e-tile == head)
        for w_sb, dstT in ((wq_sb, qT), (wk_sb, kT)):
            for h in range(H_CORE):
                pp = psum.tile([P, SC], F32, tag="pp", bufs=1)
                for t0 in range(0, N_DT, 4):
                    for t in range(t0, t0 + 4):
                        nc.tensor.matmul(
                            pp[:],
                            w_sb[:, t * E + h * HD: t * E + h * HD + HD],
                            xt_c[:, t * SC:(t + 1) * SC],
                            start=(t == 0), stop=(t == N_DT - 1))
                    yield
                raw = rope.tile([P, SC], BF, tag="raw")
                nc.scalar.copy(raw[:], pp[:])
                dst = dstT[:, h * BS + sc * SC: h * BS + (sc + 1) * SC]
                t1 = rope.tile([P, SC], BF, tag="t1")
                # raw * cos is SBUF-only: run it on the lightly-used Pool
                nc.gpsimd.tensor_mul(t1[:], raw[:], cos_sb[:, s_lo:s_lo + SC])
                if krsh:
                    # rotate-half via lane swap: rot[2j] = raw[2j+1],
                    # rot[2j+1] = raw[2j]; sin_sb carries the pair signs
                    xs = rope.tile([P, SC], BF, tag="xs")
                    nc.vector.stream_shuffle(xs[:], raw[:], SWAP_MASK)
                    t2 = rope.tile([P, SC], BF, tag="t2")
                    nc.vector.tensor_mul(t2[:], xs[:], sin_sb[:, s_lo:s_lo + SC])
                else:
                    # rot rides the "ps" (outproj) ring, NOT "pss": during a
                    # deferred den-finish the "pss" ring already holds two
                    # sc_ps tiles + dps, and a rot allocation there stalls
                    # the next scores matmul ~830ns
                    if os.environ.get("KROT", "ps") == "ps":
                        rot = psum.tile([P, SC], F32, tag="ps")
                    else:
                        rot = psum_s.tile([P, SC], F32, tag="pss")
                    nc.tensor.matmul(rot[:], rmat_sb[:], raw[:],
                                     start=True, stop=True)
                    t2 = rope.tile([P, SC], BF, tag="t2")
                    nc.vector.tensor_mul(t2[:], rot[:], sin_sb[:, s_lo:s_lo + SC])
                nc.vector.tensor_add(dst, t1[:], t2[:])
                yield

        # v for this s-chunk: 4 s-subtiles of 128, two per PSUM tile so each
        # eviction copy covers 512 columns
        for sp in range(SC // P // 2):
            pv = psum.tile([P, SC], F32, tag="pp", bufs=1)
            for half in range(2):
                st = sp * 2 + half
                for t0 in range(0, N_DT, 8):
                    for t in range(t0, t0 + 8):
                        nc.tensor.matmul(
                            pv[:, half * E:(half + 1) * E],
                            xt_c[:, t * SC + st * P: t * SC + (st + 1) * P],
                            wv_sb[:, t * E:(t + 1) * E],
                            start=(t == 0), stop=(t == N_DT - 1))
                    yield
            g_st = sc * (SC // P) + sp * 2  # global s-tile index
            nc.scalar.copy(v_sb[:, g_st * E:(g_st + 2) * E], pv[:])
            yield

    def emit_phase1(sc, xt_c):
        for _ in phase1_gen(sc, xt_c):
            pass

    # ---- attention per (batch, head); the softmax denominator is summed on
    # the Pool engine (KDEN=pool) so PE only does scores + AV ----
    # finishers: each chunk's reciprocal+normalize is deferred into the NEXT
    # chunk (emitted after its first step) so the chunk-end serial chain
    # den->recip->normalize on DVE never blocks the next chunk's exp/AV
    pend_fin = []

    def attention_chunk(b, h, qc, fillers=(), fillgen=None):
        # fillers: closures emitted at evenly spaced points of the k-loop
        # (used to spread out-projection work so its PSUM use and eviction
        # load drain gradually instead of in one burst). fillgen: a quantum
        # generator (phase1/outproj work) pulled once per step instead.
        fillers = list(fillers)
        qk_off = h * BS + b * S  # column offset into qT/kT
        # out_ps gets its own 2-deep ring: with deferred finishes it stays
        # live into the next unit, and sharing the "ps" ring would let a
        # later pp/pv/po allocation land on its bank and stall PE on the
        # (not yet emitted) finish -- a ~700ns stall per iter
        kops = int(os.environ.get("KOPS", "2"))
        if kops:
            out_ps = psum.tile([P, QC], F32, tag="out", bufs=kops)
        else:
            out_ps = psum.tile([P, QC], F32, tag="ps")
        if not (use_pool_den or use_dve_den):
            den_ps = psum.tile([P, QC], F32, tag="ps")
        nkt = (qc + 1) * (QC // P)
        ndiag = qc * (QC // P)  # number of full (below-diagonal) k-tiles
        if os.environ.get("KPAIR", "0") == "1":
            steps = [(j, j + 1) for j in range(0, ndiag - 1, 2)]
            if ndiag % 2:
                steps.append((ndiag - 1,))
        else:
            steps = [(j,) for j in range(ndiag)]
        steps += [(j,) for j in range(ndiag, nkt)]
        fill_at = {max(0, ((i + 1) * len(steps)) // len(fillers) - 1): f
                   for i, f in enumerate(fillers)} if fillers else {}
        acc = None

        def emit_av(js, at, q0):
            for i, j in enumerate(js):
                g_st = b * (S // P) + j
                nc.tensor.matmul(
                    out_ps[:, q0:QC],
                    v_sb[:, g_st * E + h * HD: g_st * E + (h + 1) * HD],
                    at[:, i * QC + q0:(i + 1) * QC],
                    start=(j == 0), stop=(j == nkt - 1))
                if not (use_pool_den or use_dve_den):
                    nc.tensor.matmul(
                        den_ps[:, q0:QC],
                        ones_sb[:],
                        at[:, i * QC + q0:(i + 1) * QC],
                        start=(j == 0), stop=(j == nkt - 1))

        def emit_den(js, at):
            nonlocal acc
            den_eng = nc.vector if use_dve_den else nc.gpsimd
            for i, j in enumerate(js):
                q0j = max(j - ndiag, 0) * P
                seg = at[:, i * QC + q0j:(i + 1) * QC]
                if acc is None:
                    acc = den.tile([P, QC], BF if use_dve_den else F32,
                                   tag="accd" if use_dve_den else "acc")
                    if q0j == 0:
                        den_eng.tensor_copy(acc[:], seg)
                    else:
                        den_eng.memset(acc[:], 0.0)
                        den_eng.tensor_add(acc[:, q0j:], acc[:, q0j:], seg)
                else:
                    den_eng.tensor_add(acc[:, q0j:], acc[:, q0j:], seg)

        # software-pipelined by one k-tile (KPIPE=1): scores(t+1) is enqueued
        # on PE before AV(t), so PE never waits on ACT's exp (532ns > the
        # 464ns PE step) -- each such wait is a >100ns gap that also drops
        # the PE clock to pstate-mid for the next 3us of work
        kpipe = os.environ.get("KPIPE", "0") == "1"
        pend_av = None
        for si, js in enumerate(steps):
            pair = len(js) == 2
            at = att.tile([P, 2 * QC] if pair else [P, QC], BF,
                          tag="at2" if pair else "at", bufs=int(os.environ.get("KAT","4")))
            sc_ps = psum_s.tile([P, 2 * QC] if pair else [P, QC], F32,
                                tag="pss")
            di = js[0] - ndiag
            q0 = max(di, 0) * P  # valid q suffix start (0 for paired tiles)
            for i, j in enumerate(js):
                nc.tensor.matmul(
                    sc_ps[:, i * QC + q0:(i + 1) * QC],
                    kT[:, qk_off + j * P: qk_off + (j + 1) * P],
                    qT[:, qk_off + qc * QC + q0: qk_off + (qc + 1) * QC],
                    start=True, stop=True)
            nc.scalar.activation(at[:, q0:], sc_ps[:, q0:], Exp, scale=SCALE)
            if di >= 0:
                tri_eng = nc.gpsimd if os.environ.get("KTRI", "dve") == "pool" else nc.vector
                tri_eng.tensor_mul(at[:, q0:q0 + P],
                                   at[:, q0:q0 + P], tri_sb[:])
            if pend_av is not None:
                emit_av(*pend_av)
                if use_pool_den or use_dve_den:
                    emit_den(pend_av[0], pend_av[1])
            pend_av = (js, at, q0)
            if si == min(1, len(steps) - 1) and kfin and pend_fin:
                for fin in pend_fin:
                    fin()
                pend_fin.clear()
            if fillgen is not None:
                # one projection/outproj quantum between scores and AV: PE
                # stays dense while ACT computes the exp this AV needs
                next(fillgen, None)
            if not kpipe:
                emit_av(*pend_av)
                if use_pool_den or use_dve_den:
                    emit_den(pend_av[0], pend_av[1])
                pend_av = None
            if si in fill_at:
                fill_at[si]()
        if pend_av is not None:
            emit_av(*pend_av)
            if use_pool_den or use_dve_den:
                emit_den(pend_av[0], pend_av[1])

        def finish(out_ps=out_ps, b=b, h=h, qc=qc, acc=acc, split=False):
            if use_dve_den:
                dps = psum_s.tile([P, QC], F32, tag="pss")
                nc.tensor.matmul(dps[:], ones_sb[:], acc[:],
                                 start=True, stop=True)
            elif use_pool_den:
                # single bf16 rounding of the final sums (+-0.2% on den), then
                # a 1-cycle/row bf16 matmul does the partition reduction
                acc_bf = den.tile([P, QC], BF, tag="accb")
                nc.gpsimd.tensor_copy(acc_bf[:], acc[:])
                dps = psum_s.tile([P, QC], F32, tag="pss")
                nc.tensor.matmul(dps[:], ones_sb[:], acc_bf[:],
                                 start=True, stop=True)
            else:
                dps = den_ps
            rec = nrm.tile([P, QC], F32, tag="rec")
            nc.vector.reciprocal_approx_fast(rec[:], dps[:])
            dst = aoT[:, (b * H_CORE + h) * S + qc * QC:
                      (b * H_CORE + h) * S + (qc + 1) * QC]
            if split:
                # kernel tail: normalize per q-subtile and launch that
                # subtile's out-projection slab immediately, pipelining the
                # final outproj/evict/DMA chain with the normalization
                for k in range(QC // P):
                    nc.vector.tensor_mul(dst[:, k * P:(k + 1) * P],
                                         out_ps[:, k * P:(k + 1) * P],
                                         rec[:, k * P:(k + 1) * P])
                    outproj_st(b, qc * (QC // P) + k, tail=True)
            else:
                nc.vector.tensor_mul(dst, out_ps[:], rec[:])
        if kfin:
            pend_fin.append(finish)
        else:
            finish()

    obf = os.environ.get("KOBF", "1") == "1"

    # b0 outproj fillers ride inside projection-heavy iters where ACT has
    # ~15-19us slack per iter (exp alone does not saturate it there), so
    # their evictions go to ACT; b1 fillers run in the att-only endgame
    # where ACT's exp is the pacer, so they go to DVE
    evict_mode = os.environ.get("KEVE", "v")

    def outproj_st(b, st, tail=False, ecs=None, evict_alt=False):
        # one 128-row slab of batch b's output, all 4 e-chunks; partials are
        # written bf16 (KOBF=1) -- the host sums 8 partials in f32, so the
        # single rounding costs ~4e-4 max-rel while halving output DMA bytes
        for ec in (range(D // SC) if ecs is None else ecs):
            # at the tail the "pp" (projection) ring is free: alternate po
            # between "ps" and "pp" for an effective 3-deep ring so the po
            # matmuls never wait on the eviction pipeline
            if tail and ec % 2:
                po = psum.tile([P, SC], F32, tag="pp", bufs=1)
            else:
                po = psum.tile([P, SC], F32, tag="ps")
            for h in range(H_CORE):
                lhsT = aoT[:, (b * H_CORE + h) * S + st * P:
                           (b * H_CORE + h) * S + (st + 1) * P]
                nc.tensor.matmul(
                    po[:],
                    lhsT,
                    wo_sb[:, h * D + ec * SC: h * D + (ec + 1) * SC],
                    start=(h == 0), stop=(h == H_CORE - 1))
            o_sb = outp.tile([P, SC], BF if obf else F32, tag="o")
            if tail or evict_alt:
                e = nc.scalar if ec % 2 == 0 else nc.vector
            elif evict_mode == "b":
                e = nc.scalar if b == 0 else nc.vector
            else:
                e = nc.vector
            if e is nc.scalar:
                nc.scalar.copy(o_sb[:], po[:])
            else:
                nc.vector.tensor_copy(o_sb[:], po[:])
            if tail:
                # nothing left to hide the drain under: spread the final
                # DMAs across four engine queues instead of serializing
                # ~620ns descriptor-gens on sync
                dma_eng = (nc.sync, nc.scalar, nc.sync, nc.scalar)[ec % 4]
            elif os.environ.get("KODM", "1") == "1":
                # alternate rings: the sync ring also carries the 14MB xt
                # stream at ~100GB/s per ring -- putting all 15MB of output
                # on it too (~83GB/s combined) starves late xt chunk loads
                dma_eng = nc.sync if ec % 2 == 0 else nc.scalar
            else:
                dma_eng = nc.sync
            dma_eng.dma_start(
                out_d[b * S + st * P: b * S + (st + 1) * P,
                      ec * SC:(ec + 1) * SC],
                o_sb[:])

    def outproj_fillers(b, qc, tail=False):
        return [lambda st=st: outproj_st(b, st, tail)
                for st in range(qc * (QC // P), (qc + 1) * (QC // P))]

    xt_split = os.environ.get("KXTS", "0") == "1"

    def load_xt_chunk(sc, eng, pieces=4):
        # KXTS=1: alternate pieces across both HWDGE queues -- phase 1 is
        # xt-supply-limited and the weights queue is idle after startup
        xt_c = xpool.tile([P, N_DT * SC], BF, tag="xt")
        step = N_DT // pieces
        for pi, t0_ in enumerate(range(0, N_DT, step)):
            e = (nc.scalar if pi % 2 else nc.sync) if xt_split else eng
            e.dma_start(xt_c[:, t0_ * SC:(t0_ + step) * SC],
                        xt_d[sc][:, t0_ * SC:(t0_ + step) * SC])
        return xt_c

    def outproj_gen(b, qc, evict_alt=False, fine=False):
        # per-(st, ec-pair) quanta: ~1us of PE each; fine=True emits per-ec
        # (~0.5us) quanta -- used in the last iter where these are the only
        # PE filler available for 32 attention steps
        for st in range(qc * (QC // P), (qc + 1) * (QC // P)):
            if fine:
                for ec in range(D // SC):
                    outproj_st(b, st, ecs=(ec,), evict_alt=evict_alt)
                    yield
            else:
                for ec0 in range(0, D // SC, 2):
                    outproj_st(b, st, ecs=(ec0, ec0 + 1), evict_alt=evict_alt)
                    yield

    def drive():
        if kilv == "5":
            # emission-interleaved: each iter runs one attention unit while
            # weaving the NEXT projection chunk's matmuls (and the previous
            # unit's out-projection) into its steps as ~1us PE quanta. The
            # projection chunk r is independent of attention unit r-1, so
            # every att step gets gap-free PE filler work.
            units = [(0, qc) for qc in range(N_QC)] + \
                    [(1, qc) for qc in range(N_QC)]
            for q_ in phase1_gen(0, xt_c0):
                pass
            for r in range(1, N_SC + 1):
                gens = []
                if r < N_SC:
                    xt_c = load_xt_chunk(r, xt_eng)
                    gens.append(phase1_gen(r, xt_c))
                if r >= 2:
                    # in the last iter (no projection work left) use fine
                    # quanta so all 32 att steps get filler, and split the
                    # eviction load ACT/DVE so the final den adds are not
                    # queued behind ~11us of DVE evictions
                    gens.append(outproj_gen(*units[r - 2],
                                            evict_alt=(r >= N_SC - 1),
                                            fine=(r == N_SC)))
                fill = (q for g in gens for q in g)
                b, qc = units[r - 1]
                attention_chunk(b, 0, qc, fillgen=fill)
                attention_chunk(b, 1, qc, fillgen=fill)
                for _ in fill:  # drain leftover quanta densely
                    pass
            if pend_fin:
                for fin in pend_fin[:-1]:
                    fin()
                pend_fin[-1](split=True)  # tail finish + outproj pipelined
                pend_fin.clear()
            else:
                for fl in outproj_fillers(B - 1, N_QC - 1, tail=True):
                    fl()
            return
        if kilv == "4":
            # fully interleaved: attention unit u (= (b, qc)) runs right
            # after projection chunk b*4+qc exists, so ACT/DVE-heavy
            # attention overlaps PE-heavy projections for the whole kernel
            # instead of piling up in a pure-attention endgame. outproj(u)
            # fillers ride inside unit u+1's chunks; the 2nd-to-last unit's
            # outproj is emitted inside the last p1 iter (which still has
            # projection work to hide its eviction load), keeping the final
            # iter's DVE load under PE.
            units = [(0, qc) for qc in range(N_QC)] + \
                    [(1, qc) for qc in range(N_QC)]
            emit_phase1(0, xt_c0)
            for r in range(1, N_SC + 1):
                xt_c = None
                if r < N_SC:
                    xt_c = load_xt_chunk(r, xt_eng)
                b, qc = units[r - 1]
                if r >= 2 and r < N_SC:
                    f = outproj_fillers(*units[r - 2])
                    f1, f2 = f[:2], f[2:]
                else:
                    f1, f2 = [], []
                attention_chunk(b, 0, qc, fillers=f1)
                attention_chunk(b, 1, qc, fillers=f2)
                if r == N_SC:
                    break
                if r == N_SC - 1:
                    # 2nd-to-last unit's outproj before the final p1 chunk
                    # (hidden under its projection load) so the last iter's
                    # DVE sees only den/norm work; must flush deferred
                    # finishes first -- they write the aoT this reads
                    for fin in pend_fin:
                        fin()
                    pend_fin.clear()
                    for fl in outproj_fillers(*units[r - 1]):
                        fl()
                emit_phase1(r, xt_c)
            for fin in pend_fin:
                fin()
            pend_fin.clear()
            for fl in outproj_fillers(B - 1, N_QC - 1, tail=True):
                fl()
            return
        if kilv == "2":
            # attention unit (b, qc) r-1 is ready after projection chunk r-1;
            # weave it before projection chunk r so PE always has DMA-free
            # work while the next x chunk streams in
            units = [(b, qc) for b in range(B) for qc in range(N_QC)]
            emit_phase1(0, xt_c0)
            for r in range(1, N_SC + 1):
                xt_c = None
                if r < N_SC:
                    xt_c = load_xt_chunk(r, xt_eng)
                b, qc = units[r - 1]
                f = outproj_fillers(*units[r - 2]) if r >= 2 else []
                attention_chunk(b, 0, qc)
                attention_chunk(b, 1, qc, fillers=f)
                if r < N_SC:
                    emit_phase1(r, xt_c)
        elif kilv == "1":
            # batch 0 projections first, then batch 1 projections interleaved
            # with batch 0 attention
            for sc in range(N_SC // B):
                emit_phase1(sc, xt_c0 if sc == 0 else load_xt_chunk(sc, nc.sync))
            for qc in range(N_QC):
                emit_phase1(N_SC // B + qc, load_xt_chunk(N_SC // B + qc, nc.sync))
                attention_chunk(0, 0, qc)
                prev = (0, qc - 1)
                fillers = outproj_fillers(*prev) if prev[1] >= 0 else ()
                attention_chunk(0, 1, qc, fillers=fillers)
            for qc in range(N_QC):
                attention_chunk(1, 0, qc)
                prev = (1, qc - 1) if qc > 0 else (0, N_QC - 1)
                attention_chunk(1, 1, qc, fillers=outproj_fillers(*prev))
        else:
            for sc in range(N_SC):
                emit_phase1(sc, xt_c0 if sc == 0 else load_xt_chunk(sc, nc.sync))
            for b in range(B):
                for qc in range(N_QC):
                    attention_chunk(b, 0, qc)
                    prev = (b, qc - 1) if qc > 0 else (b - 1, N_QC - 1)
                    fillers = outproj_fillers(*prev) if prev[0] >= 0 else ()
                    attention_chunk(b, 1, qc, fillers=fillers)
        for fin in pend_fin:
            fin()
        pend_fin.clear()
        for fl in outproj_fillers(B - 1, N_QC - 1, tail=True):
            fl()
    drive()
    warm_close()


def _rope_tables():
    """cos/sin tables exactly matching the reference's indexing quirk."""
    inv_freq = (1.0 / (ROPE_BASE ** (np.arange(0, HD, 2, dtype=np.float32) / HD)))
    t = np.arange(S, dtype=np.float32)
    freqs = np.outer(t, inv_freq)                       # [S, 64]
    emb = np.concatenate([freqs, freqs], axis=1)        # [S, 128]
    cos_part = np.cos(emb)[:, ::2]                      # [S, 64]
    sin_part = np.sin(emb)[:, 1::2]                     # [S, 64]
    # COS[d, s] = cos_part[s, d // 2]
    cos = cos_part.T[np.repeat(np.arange(HD // 2), 2)]  # [128, S]
    sin = sin_part.T[np.repeat(np.arange(HD // 2), 2)]
    return np.ascontiguousarray(cos), np.ascontiguousarray(sin)


def _pack_dtile_major(wt):
    """[D, E] (d, e) -> [128, N_DT * E]: row p holds [t, e] contiguously."""
    d, e = wt.shape
    return np.ascontiguousarray(
        wt.reshape(d // P, P, e).transpose(1, 0, 2).reshape(P, (d // P) * e))


def _host_prep(x, wq, wk, wv, wo):
    """Build the per-core input maps (SBUF-layout packed, bf16)."""
    bf = BF_NP
    xt = x.reshape(BS, D).T.astype(bf)                  # [D, BS]
    # pack to [N_SC, 128, N_DT*SC]: chunk sc, partition p -> (t, s) contiguous
    xt = np.ascontiguousarray(
        xt.reshape(N_DT, P, N_SC, SC).transpose(2, 1, 0, 3).reshape(
            N_SC, P, N_DT * SC))
    cos, sin = _rope_tables()
    # sign-baked sin for the stream_shuffle rope path: row 2j gets -sin
    # (pairs with the swapped-in odd lane), row 2j+1 gets +sin
    sign = np.where(np.arange(P) % 2 == 0, -1.0, 1.0)[:, None].astype(np.float32)
    sins = (sin * sign).astype(bf)
    cos = cos.astype(bf)
    sin = sin.astype(bf)
    rmat = np.zeros((P, P), dtype=np.float32)           # R^T for rot = R @ q
    idx = np.arange(0, P, 2)
    rmat[idx + 1, idx] = -1.0                           # R^T[2j+1, 2j] = -1
    rmat[idx, idx + 1] = 1.0                            # R^T[2j, 2j+1] = +1
    rmat = rmat.astype(bf)
    tri = np.triu(np.ones((P, P), dtype=np.float32)).astype(bf)

    in_maps = []
    for c in range(N_CORES):
        lo, hi = c * E, (c + 1) * E
        in_maps.append({
            "xt": xt,
            "wqt": _pack_dtile_major(wq[lo:hi].T.astype(bf)),
            "wkt": _pack_dtile_major(wk[lo:hi].T.astype(bf)),
            "wvt": _pack_dtile_major(wv[lo:hi].T.astype(bf)),
            "wot": _pack_dtile_major(wo[:, lo:hi].T.astype(bf)),
            "cos": cos,
            "sin": sin,
            "sins": sins,
            "rmat": rmat,
            "tri": tri,
        })
    return in_maps


_CACHE = {}


def _get_program():
    if "nc" not in _CACHE:
        _CACHE["nc"] = _build_program()
    return _CACHE["nc"]


def _run(in_maps):
    from concourse.bass_utils import run_bass_kernel_spmd
    nc = _get_program()
    res = run_bass_kernel_spmd(nc, in_maps, core_ids=list(range(N_CORES)))
    return res


def kernel(x, wq, wk, wv, wo, attn_mask=None, **_):
    x = np.asarray(x, dtype=np.float32)
    in_maps = _host_prep(np.asarray(x, np.float32), np.asarray(wq, np.float32),
                         np.asarray(wk, np.float32), np.asarray(wv, np.float32),
                         np.asarray(wo, np.float32))
    res = _run(in_maps)
    out = np.zeros((BS, D), dtype=np.float32)
    for c in range(N_CORES):
        out += np.asarray(res.results[c]["out"], dtype=np.float32)
    return out.reshape(B, S, D)


if __name__ == "__main__":
    t0 = time.time()
    _get_program()
    print(f"program build: {time.time() - t0:.1f}s")

